# revision 35
# baseline (speedup 1.0000x reference)
"""AdaptiveConv3D Trainium2 kernel — Winograd F(2,3) along depth.

Host prep computes the style modulation exactly (scale/shift/kmod are
tiny GEMMs), modulates x, and marshals the input into Winograd F(2,3)
depth-transformed planes: for each output slice pair (d0, d0+1)
  t1 = x[d0-1] - x[d0+1]   t2 = x[d0] + x[d0+1]
  t3 = x[d0+1] - x[d0]     t4 = x[d0] - x[d0+2]
shipped as zero-padded 49-pitch bf16 slices (channel-duplicated across
both SBUF partition halves). The per-sample modulated weights are
transformed along kd into 4 product sets ghat_k = G@w with
G = [[1,0,0],[.5,.5,.5],[.5,-.5,.5],[0,0,1]].

Device kernel (per core = sample b, depth half): per pair, 36
2D-shifted matmuls (9 taps x 4 products) accumulate four PSUM banks
M1..M4 per 10-row tile; outputs combine linearly on scalar/DVE:
  y[d0]   = M1 + M2 + M3        y[d0+1] = M2 - M3 - M4
This is 1.5x fewer PE column-cycles than direct 27-tap conv (36*N vs
54*N per pair). Products pair across PE halves (M1/M3 lower, M2/M4
upper) so two 64-deep matmuls stream concurrently per slot; evicts
start after the M1/M2 half of each tile to overlap the M3/M4 streams.
Engine FIFOs stay homogeneous (plane DMAs on pool/sync queues, psum
copies on scalar, psum adds on DVE, stores on scalar — sync for the
last pair's per-tile stores) so no eviction convoys behind a waiting
DMA issue. Startup ships pair-0 planes/weights in row/block chunks
whose issue points are threaded through the pair-0 emission, so each
matmul group's whole-tile dep gates only on the chunk it needs.

Sharding: 8 cores = 4 samples x 2 depth halves; no collectives (depth
halos are zero-padded host-side, pairs never straddle cores).
"""

import os
import numpy as np
import ml_dtypes

import concourse.bass as bass
import concourse.mybir as mybir
import concourse.tile as tile
from concourse import bacc
from concourse.bass import ds
from concourse.bass_utils import run_bass_kernel_spmd

F32 = mybir.dt.float32
BF16 = mybir.dt.bfloat16
NPBF16 = ml_dtypes.bfloat16

# Problem shape (hardcoded per spec).
B, CIN, COUT, KK, SDIM = 4, 64, 128, 3, 512
D = H = W = 48

# Per-core geometry.
TD = 24            # output depth slices per core
NP = TD // 2       # winograd depth pairs per core (12)
PW = 49            # padded row pitch (48 data + 1 zero)
S1 = PW * PW       # padded slice size (2401) = 48 rows + 49-col zero tail
S1P = S1 + 63      # t-plane pitch (63-col zero gap covers +/-50 shifts)
Z0 = 64            # lead margin (zeroed)
TAIL = 64          # tail margin (zeroed)
TPN = 3            # t-plane ring depth in pairs
TPCOLS = Z0 + 2 * S1P + TAIL   # one ring slot: planes (t1,t2) or (t3,t4)
OS = H * W         # compact output slice size (2304)
TROWS = [10, 10, 10, 10, 8]   # output rows per tile (5 tiles/slice)
NB = 18            # weight blocks: i<9 -> (P1[i], P2[i]), else (P3, P4)

_D2D = [(j // 3 - 1) * PW + (j % 3 - 1) for j in range(9)]

last_exec_time_ns = None
last_results = None
_cache = {}


def _build_nc():
    nc = bacc.Bacc("TRN2", target_bir_lowering=False, debug=False, num_devices=8)

    # tp[p, k]: winograd plane k of pair p, padded 49-pitch
    tp = nc.dram_tensor("tp", [NP, 4, 128, S1], BF16, kind="ExternalInput")
    wt = nc.dram_tensor("wt", [128, NB * COUT], BF16, kind="ExternalInput")
    out = nc.dram_tensor("out", [COUT, TD, OS], BF16, kind="ExternalOutput")

    IDENT = mybir.ActivationFunctionType.Identity

    with tile.TileContext(nc) as tc:
        with tc.tile_pool(name="const", bufs=1) as const:
            # per-ring-slot tiles: dep tracking is whole-tile, so a shared
            # slab would serialize pair p's matmuls behind pair p+2's DMAs
            wtf = const.tile([128, NB * 128], BF16)
            tpA_r = [
                const.tile([128, TPCOLS], BF16, name=f"tpA{q}")
                for q in range(TPN)
            ]
            tpB_r = [
                const.tile([128, TPCOLS], BF16, name=f"tpB{q}")
                for q in range(TPN)
            ]
            ringA = const.tile([128, 2 * OS], BF16)  # even local slices
            ringB = const.tile([128, 2 * OS], BF16)  # odd local slices

            ps1 = tc.alloc_tile_pool(name="ps1", bufs=2, space="PSUM")
            ps2 = tc.alloc_tile_pool(name="ps2", bufs=2, space="PSUM")
            ps3 = tc.alloc_tile_pool(name="ps3", bufs=2, space="PSUM")
            ps4 = tc.alloc_tile_pool(name="ps4", bufs=2, space="PSUM")

            def pbase(k):
                return Z0 + k * S1P

            def ldA(p, eng):
                q = p % TPN
                eng.dma_start(tpA_r[q][:, ds(pbase(0), S1)], tp[p, 0])
                eng.dma_start(tpA_r[q][:, ds(pbase(1), S1)], tp[p, 1])

            def ldB(p, eng):
                q = p % TPN
                eng.dma_start(tpB_r[q][:, ds(pbase(0), S1)], tp[p, 2])
                eng.dma_start(tpB_r[q][:, ds(pbase(1), S1)], tp[p, 3])

            def mov(t, half, off, R):
                v = t[half * 64: half * 64 + 64, off: off + R * PW]
                return v.rearrange("p (r c) -> p r c", c=PW)[:, :, 0:48]

            def tstate(p, tt, r0=None, R=None):
                R = TROWS[tt] if R is None else R
                r0 = tt * 10 if r0 is None else r0
                nt = R * 48
                return {
                    "p": p, "tt": tt, "R": R, "nt": nt, "r0": r0,
                    "toff": r0 * PW,
                    "rA": ringA[:, ds((p % 2) * OS + r0 * 48, nt)],
                    "rB": ringB[:, ds((p % 2) * OS + r0 * 48, nt)],
                }

            def runA(st, interleave=()):
                # M1/M2 product streams (planes t1/t2 across PE halves)
                p, R, nt, toff = st["p"], st["R"], st["nt"], st["toff"]
                tpA = tpA_r[p % TPN]
                m1 = ps1.tile([128, 480], F32, name="m1")
                m2 = ps2.tile([128, 480], F32, name="m2")
                st["m1"], st["m2"] = m1, m2
                il = dict(interleave)
                for i in range(9):
                    o = toff + _D2D[i]
                    nc.tensor.matmul(
                        m1[:, 0:nt], wtf[0:64, ds(i * 128, 128)],
                        mov(tpA, 0, pbase(0) + o, R),
                        start=(i == 0), stop=(i == 8))
                    nc.tensor.matmul(
                        m2[:, 0:nt], wtf[64:128, ds(i * 128, 128)],
                        mov(tpA, 1, pbase(1) + o, R),
                        start=(i == 0), stop=(i == 8))
                    if i in il:
                        il[i]()

            def ev1(st):
                # overlaps the M3/M4 streams
                nt = st["nt"]
                nc.scalar.activation(st["rA"], st["m1"][:, 0:nt], IDENT)
                nc.vector.tensor_add(st["rA"], st["rA"], st["m2"][:, 0:nt])
                nc.scalar.activation(st["rB"], st["m2"][:, 0:nt], IDENT)

            def runB(st, interleave=()):
                p, R, nt, toff = st["p"], st["R"], st["nt"], st["toff"]
                tpB = tpB_r[p % TPN]
                m3 = ps3.tile([128, 480], F32, name="m3")
                m4 = ps4.tile([128, 480], F32, name="m4")
                st["m3"], st["m4"] = m3, m4
                il = dict(interleave)
                for i in range(9):
                    o = toff + _D2D[i]
                    nc.tensor.matmul(
                        m3[:, 0:nt], wtf[0:64, ds((9 + i) * 128, 128)],
                        mov(tpB, 0, pbase(0) + o, R),
                        start=(i == 0), stop=(i == 8))
                    nc.tensor.matmul(
                        m4[:, 0:nt], wtf[64:128, ds((9 + i) * 128, 128)],
                        mov(tpB, 1, pbase(1) + o, R),
                        start=(i == 0), stop=(i == 8))
                    if i in il:
                        il[i]()

            def ev2(st):
                p, tt, nt = st["p"], st["tt"], st["nt"]
                if p == NP - 1:
                    # last pair: rB closes the critical path -> finish and
                    # store it first; per-tile stores ride the idle sync
                    # queue so scalar's FIFO holds only the ev1 copies
                    ro = st["r0"] * 48
                    nc.vector.tensor_sub(st["rB"], st["rB"], st["m3"][:, 0:nt])
                    nc.vector.tensor_sub(st["rB"], st["rB"], st["m4"][:, 0:nt])
                    nc.sync.dma_start(
                        out[:, 2 * p + 1][:, ds(ro, nt)], st["rB"])
                    nc.vector.tensor_add(st["rA"], st["rA"], st["m3"][:, 0:nt])
                    nc.sync.dma_start(
                        out[:, 2 * p][:, ds(ro, nt)], st["rA"])
                    return
                nc.vector.tensor_add(st["rA"], st["rA"], st["m3"][:, 0:nt])
                nc.vector.tensor_sub(st["rB"], st["rB"], st["m3"][:, 0:nt])
                nc.vector.tensor_sub(st["rB"], st["rB"], st["m4"][:, 0:nt])
                if tt == 4:
                    nc.scalar.dma_start(
                        out[:, 2 * p], ringA[:, ds((p % 2) * OS, OS)])
                    nc.scalar.dma_start(
                        out[:, 2 * p + 1], ringB[:, ds((p % 2) * OS, OS)])

            def emit_pairtile(p, tt, ilA=(), ilB=(), r0=None, R=None):
                st = tstate(p, tt, r0=r0, R=R)
                runA(st, ilA)
                ev1(st)
                runB(st, ilB)
                ev2(st)

            with tc.tile_pool(name="prep", bufs=1) as prep:
                del prep
                # Startup: pair-0 planes ship in three row-chunks
                # (c1 = rows 0-12 covers tile 0, c2 = rows 13-24 covers
                # tile 1, c3 = rest) with issue points threaded through
                # the pair-0 emission so each matmul group's whole-tile
                # dep gates only on the chunk it needs; weights ship in
                # 3 block-chunks the same way.
                C1, C2 = 13 * PW, 25 * PW

                def ldc(eng, t, k, pk, c0, c1):
                    eng.dma_start(
                        t[:, ds(pbase(k) + c0, c1 - c0)],
                        tp[0, pk][:, ds(c0, c1 - c0)])

                # zero the t-plane margins/gaps once (planes land whole)
                for t in tpA_r + tpB_r:
                    nc.vector.memset(t[:, 0:Z0], 0.0)
                    for k in range(2):
                        nc.vector.memset(
                            t[:, ds(Z0 + k * S1P + S1, S1P - S1)], 0.0)
                    nc.vector.memset(t[:, ds(Z0 + 2 * S1P, TAIL)], 0.0)
                nc.scalar.dma_start(
                    wtf[:, ds(0, 3 * 128)], wt[:, ds(0, 3 * 128)])
                ldc(nc.sync, tpA_r[0], 0, 0, 0, C1)      # t1 c1
                ldc(nc.sync, tpA_r[0], 1, 1, 0, C1)      # t2 c1
                ldc(nc.gpsimd, tpB_r[0], 0, 2, 0, C1)    # t3 c1
                ldc(nc.gpsimd, tpB_r[0], 1, 3, 0, C1)    # t4 c1

            emit_pairtile(
                0, 0,
                ilA=[
                    (2, lambda: nc.sync.dma_start(
                        wtf[:, ds(3 * 128, 3 * 128)],
                        wt[:, ds(3 * 128, 3 * 128)])),
                    (4, lambda: nc.scalar.dma_start(
                        wtf[:, ds(6 * 128, 3 * 128)],
                        wt[:, ds(6 * 128, 3 * 128)])),
                    (6, lambda: nc.scalar.dma_start(
                        wtf[:, ds(9 * 128, 9 * 128)],
                        wt[:, ds(9 * 128, 9 * 128)])),
                    (8, lambda: (
                        ldc(nc.sync, tpA_r[0], 0, 0, C1, C2),
                        ldc(nc.sync, tpA_r[0], 1, 1, C1, C2))),
                ],
                ilB=[
                    (2, lambda: (
                        ldc(nc.gpsimd, tpB_r[0], 0, 2, C1, C2),
                        ldc(nc.gpsimd, tpB_r[0], 1, 3, C1, C2))),
                    (6, lambda: (
                        ldc(nc.gpsimd, tpB_r[0], 0, 2, C2, S1),
                        ldc(nc.gpsimd, tpB_r[0], 1, 3, C2, S1))),
                ])
            emit_pairtile(
                0, 1,
                ilB=[(2, lambda: (
                    ldc(nc.sync, tpA_r[0], 0, 0, C2, S1),
                    ldc(nc.sync, tpA_r[0], 1, 1, C2, S1)))])
            emit_pairtile(0, 2, ilA=[(8, lambda: ldA(1, nc.sync))],
                          ilB=[(8, lambda: ldB(1, nc.gpsimd))])
            emit_pairtile(0, 3, ilA=[(8, lambda: ldA(2, nc.gpsimd))],
                          ilB=[(8, lambda: ldB(2, nc.sync))])
            emit_pairtile(0, 4)

            for p in range(1, NP):
                ilA1, ilB2 = (), ()
                if p + 2 < NP:
                    ilA1 = [(8, (lambda pp: lambda: ldA(pp, nc.gpsimd))(p + 2))]
                    ilB2 = [(8, (lambda pp: lambda: ldB(pp, nc.sync))(p + 2))]
                emit_pairtile(p, 0)
                emit_pairtile(p, 1, ilA=ilA1)
                emit_pairtile(p, 2, ilB=ilB2)
                emit_pairtile(p, 3)
                emit_pairtile(p, 4)

            ps4.release()
            ps3.release()
            ps2.release()
            ps1.release()

    nc.compile()
    return nc


def _host_prep(x, style, weight, w_scale, b_scale, w_shift, b_shift,
               w_kmod, b_kmod):
    """Exact style modulation + winograd transforms + layout."""
    scale = style @ w_scale.T + b_scale          # [B, CIN]
    shift = style @ w_shift.T + b_shift          # [B, CIN]
    kmod = (style @ w_kmod.T + b_kmod).reshape(B, CIN, KK, KK, KK)
    wmod = weight[None] * (1.0 + kmod[:, None])  # [B, COUT, CIN, 3,3,3]
    G = np.array([[1, 0, 0], [.5, .5, .5], [.5, -.5, .5], [0, 0, 1]],
                 np.float32)
    # ghat[b, k, cout, cin, kh, kw]
    ghat = np.einsum("kd,boidhw->bkoihw", G, wmod).astype(NPBF16)

    xm = (x * (1.0 + scale)[:, :, None, None, None]
          + shift[:, :, None, None, None])       # [B, CIN, D, H, W] f32

    in_maps = []
    for core in range(8):
        b, half = core // 2, core % 2
        d0 = TD * half
        # depth-padded modulated sample: index s = local d + 1
        xp = np.zeros((TD + 3, CIN, H, W), np.float32)
        lo, hi = d0 - 1, d0 + TD + 2          # local slices -1 .. TD+1
        clo, chi = max(lo, 0), min(hi, D)
        xp[clo - lo: chi - lo] = xm[b].transpose(1, 0, 2, 3)[clo:chi]
        # winograd planes per pair: [NP, 4, CIN, H, W]
        a = xp[0: 2 * NP: 2]      # x[d0-1]
        c = xp[1: 2 * NP: 2]      # x[d0]
        e = xp[2: 2 * NP + 1: 2]  # x[d0+1]
        f = xp[3: 2 * NP + 2: 2]  # x[d0+2]
        tpl = np.stack([a - e, c + e, e - c, c - f], axis=1)
        # pad into 49-pitch, duplicate halves
        tp_arr = np.zeros((NP, 4, 128, PW, PW), NPBF16)
        tp_arr[:, :, :CIN, :48, :48] = tpl
        tp_arr[:, :, CIN:, :48, :48] = tpl
        tp_arr = tp_arr.reshape(NP, 4, 128, S1)

        wt_arr = np.zeros((NB, 128, COUT), NPBF16)
        g = ghat[b]  # [4, COUT, CIN, 3, 3]
        for i in range(9):
            jh, jw = i // 3, i % 3
            wt_arr[i, :CIN] = g[0, :, :, jh, jw].T
            wt_arr[i, CIN:] = g[1, :, :, jh, jw].T
            wt_arr[9 + i, :CIN] = g[2, :, :, jh, jw].T
            wt_arr[9 + i, CIN:] = g[3, :, :, jh, jw].T
        wt2 = np.ascontiguousarray(
            wt_arr.transpose(1, 0, 2).reshape(128, NB * COUT))
        in_maps.append({"tp": tp_arr, "wt": wt2})
    return in_maps


def kernel(x, style, weight, bias, w_scale, b_scale, w_shift, b_shift,
           w_kmod, b_kmod):
    global last_exec_time_ns, last_results
    x = np.ascontiguousarray(np.asarray(x, np.float32))
    style = np.asarray(style, np.float32)
    weight = np.asarray(weight, np.float32)
    bias = np.asarray(bias, np.float32)
    w_scale = np.asarray(w_scale, np.float32)
    b_scale = np.asarray(b_scale, np.float32)
    w_shift = np.asarray(w_shift, np.float32)
    b_shift = np.asarray(b_shift, np.float32)
    w_kmod = np.asarray(w_kmod, np.float32)
    b_kmod = np.asarray(b_kmod, np.float32)

    if "nc" not in _cache:
        _cache["nc"] = _build_nc()
    nc = _cache["nc"]

    in_maps = _host_prep(x, style, weight, w_scale, b_scale, w_shift,
                         b_shift, w_kmod, b_kmod)
    trace = bool(int(os.environ.get("KERNEL_TRACE", "0")))
    res = None
    for attempt in range(5):
        try:
            res = run_bass_kernel_spmd(
                nc, in_maps, core_ids=list(range(8)), trace=trace
            )
            break
        except Exception:
            if attempt == 4:
                raise
            import time
            time.sleep(2.0 * (attempt + 1))
    last_exec_time_ns = res.exec_time_ns
    last_results = res

    out = np.empty((B, COUT, D, H, W), np.float32)
    for core in range(8):
        b, half = core // 2, core % 2
        o = np.asarray(res.results[core]["out"]).reshape(
            COUT, TD, H, W).astype(np.float32)
        out[b, :, TD * half: TD * half + TD] = o
    if np.any(bias):
        out += bias.reshape(1, COUT, 1, 1, 1)
    return out


# revision 36
# speedup vs baseline: 1.0060x; 1.0060x over previous
"""AdaptiveConv3D Trainium2 kernel — Winograd F(2,3) along depth.

Host prep computes the style modulation exactly (scale/shift/kmod are
tiny GEMMs), modulates x, and marshals the input into Winograd F(2,3)
depth-transformed planes: for each output slice pair (d0, d0+1)
  t1 = x[d0-1] - x[d0+1]   t2 = x[d0] + x[d0+1]
  t3 = x[d0+1] - x[d0]     t4 = x[d0] - x[d0+2]
shipped as zero-padded 49-pitch bf16 slices (channel-duplicated across
both SBUF partition halves). The per-sample modulated weights are
transformed along kd into 4 product sets ghat_k = G@w with
G = [[1,0,0],[.5,.5,.5],[.5,-.5,.5],[0,0,1]].

Device kernel (per core = sample b, depth half): per pair, 36
2D-shifted matmuls (9 taps x 4 products) accumulate four PSUM banks
M1..M4 per 10-row tile; outputs combine linearly on scalar/DVE:
  y[d0]   = M1 + M2 + M3        y[d0+1] = M2 - M3 - M4
This is 1.5x fewer PE column-cycles than direct 27-tap conv (36*N vs
54*N per pair). Products pair across PE halves (M1/M3 lower, M2/M4
upper) so two 64-deep matmuls stream concurrently per slot; evicts
start after the M1/M2 half of each tile to overlap the M3/M4 streams.
Engine FIFOs stay homogeneous (plane DMAs on pool/sync queues, psum
copies on scalar, psum adds on DVE, stores on scalar — sync for the
last pair's per-tile stores) so no eviction convoys behind a waiting
DMA issue. Startup ships pair-0 planes/weights in row/block chunks
whose issue points are threaded through the pair-0 emission, so each
matmul group's whole-tile dep gates only on the chunk it needs.

Sharding: 8 cores = 4 samples x 2 depth halves; no collectives (depth
halos are zero-padded host-side, pairs never straddle cores).
"""

import os
import numpy as np
import ml_dtypes

import concourse.bass as bass
import concourse.mybir as mybir
import concourse.tile as tile
from concourse import bacc
from concourse.bass import ds
from concourse.bass_utils import run_bass_kernel_spmd

F32 = mybir.dt.float32
BF16 = mybir.dt.bfloat16
NPBF16 = ml_dtypes.bfloat16

# Problem shape (hardcoded per spec).
B, CIN, COUT, KK, SDIM = 4, 64, 128, 3, 512
D = H = W = 48

# Per-core geometry.
TD = 24            # output depth slices per core
NP = TD // 2       # winograd depth pairs per core (12)
PW = 49            # padded row pitch (48 data + 1 zero)
S1 = PW * PW       # padded slice size (2401) = 48 rows + 49-col zero tail
S1P = S1 + 63      # t-plane pitch (63-col zero gap covers +/-50 shifts)
Z0 = 64            # lead margin (zeroed)
TAIL = 64          # tail margin (zeroed)
TPN = 3            # t-plane ring depth in pairs
TPCOLS = Z0 + 2 * S1P + TAIL   # one ring slot: planes (t1,t2) or (t3,t4)
OS = H * W         # compact output slice size (2304)
TROWS = [10, 10, 10, 10, 8]   # output rows per tile (5 tiles/slice)
NB = 18            # weight blocks: i<9 -> (P1[i], P2[i]), else (P3, P4)

_D2D = [(j // 3 - 1) * PW + (j % 3 - 1) for j in range(9)]

last_exec_time_ns = None
last_results = None
_cache = {}


def _build_nc():
    nc = bacc.Bacc("TRN2", target_bir_lowering=False, debug=False, num_devices=8)

    # tp[p, k]: winograd plane k of pair p, padded 49-pitch
    tp = nc.dram_tensor("tp", [NP, 4, 128, S1], BF16, kind="ExternalInput")
    wt = nc.dram_tensor("wt", [128, NB * COUT], BF16, kind="ExternalInput")
    out = nc.dram_tensor("out", [COUT, TD, OS], BF16, kind="ExternalOutput")

    IDENT = mybir.ActivationFunctionType.Identity

    with tile.TileContext(nc) as tc:
        with tc.tile_pool(name="const", bufs=1) as const:
            # per-ring-slot tiles: dep tracking is whole-tile, so a shared
            # slab would serialize pair p's matmuls behind pair p+2's DMAs
            wtf = const.tile([128, NB * 128], BF16)
            tpA_r = [
                const.tile([128, TPCOLS], BF16, name=f"tpA{q}")
                for q in range(TPN)
            ]
            tpB_r = [
                const.tile([128, TPCOLS], BF16, name=f"tpB{q}")
                for q in range(TPN)
            ]
            ringA = const.tile([128, 2 * OS], BF16)  # even local slices
            ringB = const.tile([128, 2 * OS], BF16)  # odd local slices

            ps1 = tc.alloc_tile_pool(name="ps1", bufs=2, space="PSUM")
            ps2 = tc.alloc_tile_pool(name="ps2", bufs=2, space="PSUM")
            ps3 = tc.alloc_tile_pool(name="ps3", bufs=2, space="PSUM")
            ps4 = tc.alloc_tile_pool(name="ps4", bufs=2, space="PSUM")

            def pbase(k):
                return Z0 + k * S1P

            def ldA(p, eng):
                q = p % TPN
                eng.dma_start(tpA_r[q][:, ds(pbase(0), S1)], tp[p, 0])
                eng.dma_start(tpA_r[q][:, ds(pbase(1), S1)], tp[p, 1])

            def ldB(p, eng):
                q = p % TPN
                eng.dma_start(tpB_r[q][:, ds(pbase(0), S1)], tp[p, 2])
                eng.dma_start(tpB_r[q][:, ds(pbase(1), S1)], tp[p, 3])

            def mov(t, half, off, R):
                v = t[half * 64: half * 64 + 64, off: off + R * PW]
                return v.rearrange("p (r c) -> p r c", c=PW)[:, :, 0:48]

            def tstate(p, tt, r0=None, R=None):
                R = TROWS[tt] if R is None else R
                r0 = tt * 10 if r0 is None else r0
                nt = R * 48
                return {
                    "p": p, "tt": tt, "R": R, "nt": nt, "r0": r0,
                    "toff": r0 * PW,
                    "rA": ringA[:, ds((p % 2) * OS + r0 * 48, nt)],
                    "rB": ringB[:, ds((p % 2) * OS + r0 * 48, nt)],
                }

            def runA(st, interleave=()):
                # M1/M2 product streams (planes t1/t2 across PE halves)
                p, R, nt, toff = st["p"], st["R"], st["nt"], st["toff"]
                tpA = tpA_r[p % TPN]
                m1 = ps1.tile([128, 480], F32, name="m1")
                m2 = ps2.tile([128, 480], F32, name="m2")
                st["m1"], st["m2"] = m1, m2
                il = dict(interleave)
                for i in range(9):
                    o = toff + _D2D[i]
                    nc.tensor.matmul(
                        m1[:, 0:nt], wtf[0:64, ds(i * 128, 128)],
                        mov(tpA, 0, pbase(0) + o, R),
                        start=(i == 0), stop=(i == 8))
                    nc.tensor.matmul(
                        m2[:, 0:nt], wtf[64:128, ds(i * 128, 128)],
                        mov(tpA, 1, pbase(1) + o, R),
                        start=(i == 0), stop=(i == 8))
                    if i in il:
                        il[i]()

            def ev1(st):
                # overlaps the M3/M4 streams
                nt = st["nt"]
                nc.scalar.activation(st["rA"], st["m1"][:, 0:nt], IDENT)
                nc.vector.tensor_add(st["rA"], st["rA"], st["m2"][:, 0:nt])
                nc.scalar.activation(st["rB"], st["m2"][:, 0:nt], IDENT)

            def runB(st, interleave=()):
                p, R, nt, toff = st["p"], st["R"], st["nt"], st["toff"]
                tpB = tpB_r[p % TPN]
                m3 = ps3.tile([128, 480], F32, name="m3")
                m4 = ps4.tile([128, 480], F32, name="m4")
                st["m3"], st["m4"] = m3, m4
                il = dict(interleave)
                for i in range(9):
                    o = toff + _D2D[i]
                    nc.tensor.matmul(
                        m3[:, 0:nt], wtf[0:64, ds((9 + i) * 128, 128)],
                        mov(tpB, 0, pbase(0) + o, R),
                        start=(i == 0), stop=(i == 8))
                    nc.tensor.matmul(
                        m4[:, 0:nt], wtf[64:128, ds((9 + i) * 128, 128)],
                        mov(tpB, 1, pbase(1) + o, R),
                        start=(i == 0), stop=(i == 8))
                    if i in il:
                        il[i]()

            def ev2(st):
                p, tt, nt = st["p"], st["tt"], st["nt"]
                if p == NP - 1:
                    # last pair: rB closes the critical path -> finish and
                    # store it first; per-tile stores ride the idle sync
                    # queue so scalar's FIFO holds only the ev1 copies
                    ro = st["r0"] * 48
                    nc.vector.tensor_sub(st["rB"], st["rB"], st["m3"][:, 0:nt])
                    nc.vector.tensor_sub(st["rB"], st["rB"], st["m4"][:, 0:nt])
                    nc.sync.dma_start(
                        out[:, 2 * p + 1][:, ds(ro, nt)], st["rB"])
                    nc.vector.tensor_add(st["rA"], st["rA"], st["m3"][:, 0:nt])
                    nc.sync.dma_start(
                        out[:, 2 * p][:, ds(ro, nt)], st["rA"])
                    return
                nc.vector.tensor_add(st["rA"], st["rA"], st["m3"][:, 0:nt])
                nc.vector.tensor_sub(st["rB"], st["rB"], st["m3"][:, 0:nt])
                nc.vector.tensor_sub(st["rB"], st["rB"], st["m4"][:, 0:nt])
                if tt == 4:
                    nc.scalar.dma_start(
                        out[:, 2 * p], ringA[:, ds((p % 2) * OS, OS)])
                    nc.scalar.dma_start(
                        out[:, 2 * p + 1], ringB[:, ds((p % 2) * OS, OS)])

            def emit_pairtile(p, tt, ilA=(), ilB=(), r0=None, R=None):
                st = tstate(p, tt, r0=r0, R=R)
                runA(st, ilA)
                ev1(st)
                runB(st, ilB)
                ev2(st)

            with tc.tile_pool(name="prep", bufs=1) as prep:
                del prep
                # Startup: pair-0 planes ship in three row-chunks
                # (c1 = rows 0-12 covers tile 0, c2 = rows 13-24 covers
                # tile 1, c3 = rest) with issue points threaded through
                # the pair-0 emission so each matmul group's whole-tile
                # dep gates only on the chunk it needs; weights ship in
                # 3 block-chunks the same way.
                C1, C2 = 13 * PW, 25 * PW

                def ldc(eng, t, k, pk, c0, c1):
                    eng.dma_start(
                        t[:, ds(pbase(k) + c0, c1 - c0)],
                        tp[0, pk][:, ds(c0, c1 - c0)])

                # zero the t-plane margins/gaps once (planes land whole)
                for t in tpA_r + tpB_r:
                    nc.vector.memset(t[:, 0:Z0], 0.0)
                    for k in range(2):
                        nc.vector.memset(
                            t[:, ds(Z0 + k * S1P + S1, S1P - S1)], 0.0)
                    nc.vector.memset(t[:, ds(Z0 + 2 * S1P, TAIL)], 0.0)
                nc.scalar.dma_start(
                    wtf[:, ds(0, 3 * 128)], wt[:, ds(0, 3 * 128)])
                ldc(nc.sync, tpA_r[0], 0, 0, 0, C1)      # t1 c1
                ldc(nc.sync, tpA_r[0], 1, 1, 0, C1)      # t2 c1
                ldc(nc.gpsimd, tpB_r[0], 0, 2, 0, C1)    # t3 c1
                ldc(nc.gpsimd, tpB_r[0], 1, 3, 0, C1)    # t4 c1

            emit_pairtile(
                0, 0,
                ilA=[
                    (2, lambda: nc.scalar.dma_start(
                        wtf[:, ds(3 * 128, 6 * 128)],
                        wt[:, ds(3 * 128, 6 * 128)])),
                    (4, lambda: nc.scalar.dma_start(
                        wtf[:, ds(9 * 128, 9 * 128)],
                        wt[:, ds(9 * 128, 9 * 128)])),
                    (8, lambda: (
                        ldc(nc.sync, tpA_r[0], 0, 0, C1, C2),
                        ldc(nc.sync, tpA_r[0], 1, 1, C1, C2))),
                ],
                ilB=[
                    (2, lambda: (
                        ldc(nc.gpsimd, tpB_r[0], 0, 2, C1, C2),
                        ldc(nc.gpsimd, tpB_r[0], 1, 3, C1, C2))),
                    (6, lambda: (
                        ldc(nc.gpsimd, tpB_r[0], 0, 2, C2, S1),
                        ldc(nc.gpsimd, tpB_r[0], 1, 3, C2, S1))),
                ])
            emit_pairtile(
                0, 1,
                ilB=[(2, lambda: (
                    ldc(nc.sync, tpA_r[0], 0, 0, C2, S1),
                    ldc(nc.sync, tpA_r[0], 1, 1, C2, S1)))])
            emit_pairtile(0, 2, ilA=[(8, lambda: ldA(1, nc.sync))],
                          ilB=[(8, lambda: ldB(1, nc.gpsimd))])
            emit_pairtile(0, 3, ilA=[(8, lambda: ldA(2, nc.gpsimd))],
                          ilB=[(8, lambda: ldB(2, nc.sync))])
            emit_pairtile(0, 4)

            for p in range(1, NP):
                ilA1, ilB2 = (), ()
                if p + 2 < NP:
                    ilA1 = [(8, (lambda pp: lambda: ldA(pp, nc.gpsimd))(p + 2))]
                    ilB2 = [(8, (lambda pp: lambda: ldB(pp, nc.sync))(p + 2))]
                emit_pairtile(p, 0)
                emit_pairtile(p, 1, ilA=ilA1)
                emit_pairtile(p, 2, ilB=ilB2)
                emit_pairtile(p, 3)
                emit_pairtile(p, 4)

            ps4.release()
            ps3.release()
            ps2.release()
            ps1.release()

    nc.compile()
    return nc


def _host_prep(x, style, weight, w_scale, b_scale, w_shift, b_shift,
               w_kmod, b_kmod):
    """Exact style modulation + winograd transforms + layout."""
    scale = style @ w_scale.T + b_scale          # [B, CIN]
    shift = style @ w_shift.T + b_shift          # [B, CIN]
    kmod = (style @ w_kmod.T + b_kmod).reshape(B, CIN, KK, KK, KK)
    wmod = weight[None] * (1.0 + kmod[:, None])  # [B, COUT, CIN, 3,3,3]
    G = np.array([[1, 0, 0], [.5, .5, .5], [.5, -.5, .5], [0, 0, 1]],
                 np.float32)
    # ghat[b, k, cout, cin, kh, kw]
    ghat = np.einsum("kd,boidhw->bkoihw", G, wmod).astype(NPBF16)

    xm = (x * (1.0 + scale)[:, :, None, None, None]
          + shift[:, :, None, None, None])       # [B, CIN, D, H, W] f32

    in_maps = []
    for core in range(8):
        b, half = core // 2, core % 2
        d0 = TD * half
        # depth-padded modulated sample: index s = local d + 1
        xp = np.zeros((TD + 3, CIN, H, W), np.float32)
        lo, hi = d0 - 1, d0 + TD + 2          # local slices -1 .. TD+1
        clo, chi = max(lo, 0), min(hi, D)
        xp[clo - lo: chi - lo] = xm[b].transpose(1, 0, 2, 3)[clo:chi]
        # winograd planes per pair: [NP, 4, CIN, H, W]
        a = xp[0: 2 * NP: 2]      # x[d0-1]
        c = xp[1: 2 * NP: 2]      # x[d0]
        e = xp[2: 2 * NP + 1: 2]  # x[d0+1]
        f = xp[3: 2 * NP + 2: 2]  # x[d0+2]
        tpl = np.stack([a - e, c + e, e - c, c - f], axis=1)
        # pad into 49-pitch, duplicate halves
        tp_arr = np.zeros((NP, 4, 128, PW, PW), NPBF16)
        tp_arr[:, :, :CIN, :48, :48] = tpl
        tp_arr[:, :, CIN:, :48, :48] = tpl
        tp_arr = tp_arr.reshape(NP, 4, 128, S1)

        wt_arr = np.zeros((NB, 128, COUT), NPBF16)
        g = ghat[b]  # [4, COUT, CIN, 3, 3]
        for i in range(9):
            jh, jw = i // 3, i % 3
            wt_arr[i, :CIN] = g[0, :, :, jh, jw].T
            wt_arr[i, CIN:] = g[1, :, :, jh, jw].T
            wt_arr[9 + i, :CIN] = g[2, :, :, jh, jw].T
            wt_arr[9 + i, CIN:] = g[3, :, :, jh, jw].T
        wt2 = np.ascontiguousarray(
            wt_arr.transpose(1, 0, 2).reshape(128, NB * COUT))
        in_maps.append({"tp": tp_arr, "wt": wt2})
    return in_maps


def kernel(x, style, weight, bias, w_scale, b_scale, w_shift, b_shift,
           w_kmod, b_kmod):
    global last_exec_time_ns, last_results
    x = np.ascontiguousarray(np.asarray(x, np.float32))
    style = np.asarray(style, np.float32)
    weight = np.asarray(weight, np.float32)
    bias = np.asarray(bias, np.float32)
    w_scale = np.asarray(w_scale, np.float32)
    b_scale = np.asarray(b_scale, np.float32)
    w_shift = np.asarray(w_shift, np.float32)
    b_shift = np.asarray(b_shift, np.float32)
    w_kmod = np.asarray(w_kmod, np.float32)
    b_kmod = np.asarray(b_kmod, np.float32)

    if "nc" not in _cache:
        _cache["nc"] = _build_nc()
    nc = _cache["nc"]

    in_maps = _host_prep(x, style, weight, w_scale, b_scale, w_shift,
                         b_shift, w_kmod, b_kmod)
    trace = bool(int(os.environ.get("KERNEL_TRACE", "0")))
    res = None
    for attempt in range(5):
        try:
            res = run_bass_kernel_spmd(
                nc, in_maps, core_ids=list(range(8)), trace=trace
            )
            break
        except Exception:
            if attempt == 4:
                raise
            import time
            time.sleep(2.0 * (attempt + 1))
    last_exec_time_ns = res.exec_time_ns
    last_results = res

    out = np.empty((B, COUT, D, H, W), np.float32)
    for core in range(8):
        b, half = core // 2, core % 2
        o = np.asarray(res.results[core]["out"]).reshape(
            COUT, TD, H, W).astype(np.float32)
        out[b, :, TD * half: TD * half + TD] = o
    if np.any(bias):
        out += bias.reshape(1, COUT, 1, 1, 1)
    return out
